# revision 1
# baseline (speedup 1.0000x reference)
"""Trainium2 Bass kernel for the FFF (fast feedforward / MoE-routing) module.

Math (per token x of dim 1024, PAR=8 trees of 255 nodes):
  logits = x @ W_in.T + b_in                      # [B, 2040]
  dec    = logits > 0
  acts   = silu(logits)
  dmap   = indicator of the 8 visited nodes per tree (root + 7 descents,
           descending by dec at the current node)
  out    = (acts * dmap) @ W_out.T                # [B, 1024]

Strategy (8 NeuronCores, data-parallel over the 8192 tokens, 1024 each):
  - GEMM1 in bf16 hi/lo split: 3 passes (hi*hi + hi*lo + lo*hi) for the
    decision-relevant node levels 0..6 (cols 0..1020), 1 pass (hi*hi) for the
    leaf level (cols 1020..2040) whose sign is never used.  PSUM accumulates
    fp32; the fp32 bias is added on the vector engine, so decision signs are
    ~fp32-accurate.
  - dmap is built level-by-level with strided vector ops in a node-major
    column layout (col = 8*node + tree): child1 = V_d * dec_d (stride-2
    upsample), child0 = V_d - child1.
  - masked acts cast to fp16, transposed 128x128 on the PE, GEMM2 in fp16
    (exact products, fp32 PSUM accumulation).
  - weight DMAs are chunked and emitted in need-order so the PE starts
    within a few us instead of waiting for the full 13.6 MB weight load.
"""

import numpy as np
import ml_dtypes

DIM = 1024
PAR = 8
DEPTH = 7
N_NODES = 255
WIDTH = PAR * N_NODES          # 2040
NODES_PAD = 2048               # pad masked-acts/W_out^T to 16*128
N_CORES = 8
TOK_PER_CORE = 1024
TT = 128                       # tokens per tile
NTILES = TOK_PER_CORE // TT    # 8
NT_W = 510                     # GEMM1 n-tile width (4 * 510 = 2040)
K_CH = DIM // 128              # 8 contraction chunks for GEMM1
C_CH = NODES_PAD // 128        # 16 contraction chunks for GEMM2
DEC_COLS = 8 * 127             # 1016: decision nodes are levels 0..6

_PROGRAM = None


def _build_program():
    import concourse.bacc as bacc
    import concourse.tile as tile
    from concourse import mybir
    from concourse.masks import make_identity
    import concourse.bass as bass

    f32 = mybir.dt.float32
    bf16 = mybir.dt.bfloat16
    f16 = mybir.dt.float16
    Alu = mybir.AluOpType
    Act = mybir.ActivationFunctionType

    nc = bacc.Bacc("TRN2", target_bir_lowering=False, debug=False,
                   num_devices=N_CORES)

    # Per-core DRAM I/O (layouts chosen so every DMA has long contiguous
    # runs); xt packs the bf16 hi/lo split as [...,0/1,...]
    xt = nc.dram_tensor("xt", [128, NTILES, 2, K_CH, TT], bf16,
                        kind="ExternalInput")
    w1_hi = nc.dram_tensor("w1_hi", [128, K_CH, WIDTH], bf16,
                           kind="ExternalInput")
    # lo-part only needed for the decision region (cols 0..1020)
    w1_lo = nc.dram_tensor("w1_lo", [128, K_CH, 2 * NT_W], bf16,
                           kind="ExternalInput")
    b1 = nc.dram_tensor("b1", [WIDTH], f32, kind="ExternalInput")
    w2 = nc.dram_tensor("w2", [128, C_CH, DIM], f16, kind="ExternalInput")
    y = nc.dram_tensor("y", [TOK_PER_CORE, DIM], f32, kind="ExternalOutput")

    with tile.TileContext(nc) as tc:
        with (
            tc.tile_pool(name="wts", bufs=1) as wts,
            tc.tile_pool(name="xts", bufs=3) as xts,
            tc.tile_pool(name="logits", bufs=2) as logits_pool,
            tc.tile_pool(name="mask", bufs=2) as mask_pool,
            tc.tile_pool(name="acts", bufs=2) as acts_pool,
            tc.tile_pool(name="out", bufs=2) as out_pool,
            tc.tile_pool(name="pl", bufs=4, space="PSUM") as pl_pool,
            tc.tile_pool(name="pt", bufs=2, space="PSUM") as pt_pool,
            tc.tile_pool(name="py", bufs=2, space="PSUM") as py_pool,
        ):
            # ---- resident weights (DMAs emitted in need-order below) ----
            w1h_sb = wts.tile([128, K_CH, WIDTH], bf16)
            w1l_sb = wts.tile([128, K_CH, 2 * NT_W], bf16)
            w2_sb = wts.tile([128, C_CH, DIM], f16)
            b1_sb = wts.tile([128, WIDTH], f32)
            ident = wts.tile([128, 128], f16)

            xt_tiles = {}

            def prefetch_xt(j, eng=None):
                xhl = xts.tile([128, 2, K_CH, TT], bf16, tag="x")
                (eng or nc.sync).dma_start(out=xhl, in_=xt[:, j, :, :, :])
                xt_tiles[j] = xhl

            # Weight DMAs chunked and emitted on the Sync engine in
            # consumption order (only Sync's HW DGE fans out over all 16
            # queues, ~400 GB/s; each dma_start dispatch costs ~0.6us).
            # x-tile prefetches ride GpSimd's slower SW DGE — their
            # deadlines are generous and this keeps Sync free for weights.
            nc.sync.dma_start(out=w1h_sb[:, 0, :], in_=w1_hi[:, 0, :])
            xhl0 = xts.tile([128, 2, K_CH, TT], bf16, tag="x")
            nc.sync.dma_start(out=xhl0[:, 0], in_=xt[:, 0, 0, :, :])
            nc.sync.dma_start(out=xhl0[:, 1], in_=xt[:, 0, 1, :, :])
            xt_tiles[0] = xhl0
            nc.sync.dma_start(out=w1l_sb[:, 0, :], in_=w1_lo[:, 0, :])
            nc.sync.dma_start(out=w1h_sb[:, 1, :], in_=w1_hi[:, 1, :])
            nc.sync.dma_start(out=w1l_sb[:, 1, :], in_=w1_lo[:, 1, :])
            for k in range(2, K_CH, 2):
                nc.sync.dma_start(out=w1h_sb[:, k:k + 2, :],
                                  in_=w1_hi[:, k:k + 2, :])
                nc.sync.dma_start(out=w1l_sb[:, k:k + 2, :],
                                  in_=w1_lo[:, k:k + 2, :])
            prefetch_xt(1)
            for c in range(0, C_CH, 4):
                nc.sync.dma_start(out=w2_sb[:, c:c + 4, :],
                                  in_=w2[:, c:c + 4, :])
            # bias broadcast rides GpSimd's SW DGE, off the weight path
            b1_bcast = bass.AP(tensor=b1, offset=0, ap=[[0, 128], [1, WIDTH]])
            nc.gpsimd.dma_start(out=b1_sb, in_=b1_bcast)
            make_identity(nc, ident)
            for c in range(C_CH):
                nc.sync.dma_start(out=w2_sb[:, c, :], in_=w2[:, c, :])

            # per-token-tile masked-acts, produced by stage A (GEMM1+mask),
            # consumed by stage B (transpose + GEMM2); 1-deep software
            # pipeline so the PE never waits on the vector-engine epilogue.
            state = {}

            def stage_a(j):
                if j not in xt_tiles:
                    prefetch_xt(j, nc.gpsimd)
                xhl = xt_tiles.pop(j)
                xh, xl = xhl[:, 0], xhl[:, 1]

                lg = logits_pool.tile([TT, WIDTH], f32, tag="lg")
                d1 = mask_pool.tile([TT, DEC_COLS], f16, tag="d1")
                vv = mask_pool.tile([TT, WIDTH], f16, tag="vv")
                ac = acts_pool.tile([TT, WIDTH], f16, tag="ac")
                mk = acts_pool.tile([TT, NODES_PAD], f16, tag="mk")

                for nt in range(4):
                    nsl = slice(nt * NT_W, (nt + 1) * NT_W)
                    pl = pl_pool.tile([TT, NT_W], f32)
                    npass = 3 if nt < 2 else 1
                    nmm = K_CH * npass
                    i = 0
                    for k in range(K_CH):
                        mms = [(xh, w1h_sb)]
                        if npass == 3:
                            mms += [(xh, w1l_sb), (xl, w1h_sb)]
                        for (xx, ww) in mms:
                            nc.tensor.matmul(
                                pl, lhsT=xx[:, k, :], rhs=ww[:, k, nsl],
                                start=(i == 0), stop=(i == nmm - 1))
                            i += 1
                    # bias add (fp32, exact) PSUM -> SBUF
                    nc.vector.tensor_tensor(lg[:, nsl], pl, b1_sb[:, nsl],
                                            Alu.add)
                    if nt == 0:
                        nc.vector.tensor_scalar(
                            d1[:, 0:NT_W], lg[:, 0:NT_W], 0.0, None,
                            Alu.is_gt)
                    elif nt == 1:
                        nc.vector.tensor_scalar(
                            d1[:, NT_W:DEC_COLS], lg[:, NT_W:DEC_COLS],
                            0.0, None, Alu.is_gt)
                    nc.scalar.activation(ac[:, nsl], lg[:, nsl], Act.Silu)

                # tree mask: V_0 = 1 at root cols; then per level
                # child1 = V_d * dec_d, child0 = V_d - child1
                nc.vector.memset(vv[:, 0:8], 1.0)
                for d in range(DEPTH):
                    ld = 8 * (1 << d)
                    c0 = 8 * ((1 << d) - 1)
                    c1 = 8 * ((1 << (d + 1)) - 1)
                    vpar = vv[:, c0:c0 + ld].rearrange("p (i t) -> p i t", t=8)
                    dpar = d1[:, c0:c0 + ld].rearrange("p (i t) -> p i t", t=8)
                    kids = vv[:, c1:c1 + 2 * ld].rearrange(
                        "p (i two t) -> p i two t", two=2, t=8)
                    nc.vector.tensor_tensor(kids[:, :, 1, :], vpar, dpar,
                                            Alu.mult)
                    nc.vector.tensor_tensor(kids[:, :, 0, :], vpar,
                                            kids[:, :, 1, :], Alu.subtract)

                # masked acts (fp16); cols 2040:2048 are zero padding so the
                # last transpose/GEMM2 chunk is a uniform 128 wide
                nc.vector.memset(mk[:, WIDTH:NODES_PAD], 0.0)
                nc.vector.tensor_tensor(mk[:, 0:1024], ac[:, 0:1024],
                                        vv[:, 0:1024], Alu.mult)
                nc.vector.tensor_tensor(mk[:, 1024:WIDTH], ac[:, 1024:WIDTH],
                                        vv[:, 1024:WIDTH], Alu.mult)
                state[j] = mk

            def stage_b(j):
                mk = state.pop(j)
                at = acts_pool.tile([128, C_CH, TT], f16, tag="at")
                # transpose in groups -> one PSUM tile -> one copy; first
                # group is a single chunk so GEMM2 can start immediately
                c = 0
                for gsz in (1, 3, 4, 4, 4):
                    pt = pt_pool.tile([128, 512], f16)
                    for i in range(gsz):
                        nc.tensor.transpose(
                            pt[:, i * 128:(i + 1) * 128],
                            mk[:, (c + i) * 128:(c + i + 1) * 128], ident)
                    nc.scalar.copy(
                        at[:, c:c + gsz, :],
                        pt[:, :gsz * 128].rearrange("p (c t) -> p c t", t=TT))
                    c += gsz
                ys = out_pool.tile([TT, DIM], f32, tag="ys")
                for h in range(2):
                    hs = slice(h * 512, (h + 1) * 512)
                    py = py_pool.tile([TT, 512], f32)
                    for c in range(C_CH):
                        nc.tensor.matmul(
                            py, lhsT=at[:, c, :], rhs=w2_sb[:, c, hs],
                            start=(c == 0), stop=(c == C_CH - 1))
                    nc.vector.tensor_copy(ys[:, hs], py)
                    nc.sync.dma_start(out=y[j * TT:(j + 1) * TT, hs],
                                      in_=ys[:, hs])

            # software pipeline: A(0), A(1), B(0), A(2), B(1), ... B(7)
            stage_a(0)
            for j in range(1, NTILES):
                stage_a(j)
                stage_b(j - 1)
            stage_b(NTILES - 1)

    nc.finalize()
    return nc


def _get_program():
    global _PROGRAM
    if _PROGRAM is None:
        _PROGRAM = _build_program()
    return _PROGRAM


def _split_hi_lo(a):
    hi = a.astype(ml_dtypes.bfloat16)
    lo = (a - hi.astype(np.float32)).astype(ml_dtypes.bfloat16)
    return hi, lo


def kernel(oldx, W_in, b_in, W_out):
    from concourse.bass_utils import run_bass_kernel_spmd

    oldx = np.asarray(oldx)
    W_in = np.asarray(W_in, dtype=np.float32)
    b_in = np.asarray(b_in, dtype=np.float32)
    W_out = np.asarray(W_out, dtype=np.float32)
    x = oldx.reshape(-1, DIM).astype(np.float32)          # [8192, 1024]

    # node-major column permutation: our col 8n+t  <-  ref col 255t+n
    i = np.arange(WIDTH)
    perm = 255 * (i % PAR) + (i // PAR)

    w1t = W_in[perm, :].T.astype(np.float32)              # [1024, 2040]
    w1t_hi, w1t_lo = _split_hi_lo(w1t)
    # [dim, width] -> [128, K_CH, WIDTH] with dim = k*128 + p
    w1_hi = np.ascontiguousarray(
        w1t_hi.reshape(K_CH, 128, WIDTH).transpose(1, 0, 2))
    w1_lo = np.ascontiguousarray(
        w1t_lo.reshape(K_CH, 128, WIDTH).transpose(1, 0, 2)[:, :, :2 * NT_W])
    b1 = np.ascontiguousarray(b_in[perm])

    w2t = np.zeros((NODES_PAD, DIM), np.float32)
    w2t[:WIDTH] = W_out.T[perm, :]
    w2 = np.ascontiguousarray(
        w2t.astype(np.float16).reshape(C_CH, 128, DIM).transpose(1, 0, 2))

    in_maps = []
    for c in range(N_CORES):
        xc = x[c * TOK_PER_CORE:(c + 1) * TOK_PER_CORE]   # [1024, 1024]
        xt_hi, xt_lo = _split_hi_lo(xc.T)                 # [dim, tok]
        # [dim, tok] -> [128, NTILES, K_CH, TT]; dim = k*128+p, tok = j*128+t
        xt_hi = xt_hi.reshape(K_CH, 128, NTILES, TT).transpose(1, 2, 0, 3)
        xt_lo = xt_lo.reshape(K_CH, 128, NTILES, TT).transpose(1, 2, 0, 3)
        xt = np.ascontiguousarray(np.stack([xt_hi, xt_lo], axis=2))
        in_maps.append({
            "xt": xt, "w1_hi": w1_hi, "w1_lo": w1_lo,
            "b1": b1, "w2": w2,
        })

    nc = _get_program()
    res = run_bass_kernel_spmd(nc, in_maps, core_ids=list(range(N_CORES)))
    out = np.concatenate([res.results[c]["y"] for c in range(N_CORES)],
                         axis=0)
    return out.reshape(oldx.shape).astype(np.float32)



# revision 7
# speedup vs baseline: 1.2195x; 1.2195x over previous
"""Trainium2 Bass kernel for the FFF (fast feedforward / MoE-routing) module.

Math (per token x of dim 1024, PAR=8 trees of 255 nodes):
  logits = x @ W_in.T + b_in                      # [B, 2040]
  dec    = logits > 0
  acts   = silu(logits)
  dmap   = indicator of the 8 visited nodes per tree (root + 7 descents,
           descending by dec at the current node)
  out    = (acts * dmap) @ W_out.T                # [B, 1024]

Strategy (8 NeuronCores, data-parallel over the 8192 tokens, 1024 each):
  - GEMM1 main pass in fp16 (exact products, fp32 PSUM accumulation) over
    all 2040 node columns.  Decision signs for the early tree levels need
    more accuracy than fp16 inputs give (logit err ~1.6e-4 flips ~0.16% of
    near-zero decisions), and a flip at depth d corrupts 7-d downstream
    nodes — so the first 256 node-major columns (levels 0..4) also get a
    bf16 correction pass (eps_x@w + x@eps_w, accumulated into the same
    PSUM group; bf16 residuals need no scaling).  Tile 0 of each core
    skips the correction so the PE is not stalled on the correction-weight
    DMA while the main weights stream in.
  - dmap is built level-by-level with strided vector ops in a node-major
    column layout (col = 8*node + tree): child1 = V_d * dec_d (stride-2
    upsample), child0 = V_d - child1.
  - masked acts cast to fp16, transposed 128x128 on the PE, GEMM2 in fp16
    (exact products, fp32 PSUM accumulation).
  - weight DMAs are chunked and emitted in need-order so the PE starts
    within ~1us instead of waiting for the full weight load.
"""

import numpy as np
import ml_dtypes

DIM = 1024
PAR = 8
DEPTH = 7
N_NODES = 255
WIDTH = PAR * N_NODES          # 2040
NODES_PAD = 2048               # pad masked-acts/W_out^T to 16*128
N_CORES = 8
TOK_PER_CORE = 1024
TT = 128                       # tokens per tile
NTILES = TOK_PER_CORE // TT    # 8
NT_W = 510                     # GEMM1 n-tile width (4 * 510 = 2040)
K_CH = DIM // 128              # 8 contraction chunks for GEMM1
C_CH = NODES_PAD // 128        # 16 contraction chunks for GEMM2
DEC_COLS = 8 * 127             # 1016: decision nodes are levels 0..6
CORR = 256                     # corrected cols (nodes 0..31 = levels 0..4)

_PROGRAM = None


def _build_program():
    import concourse.bacc as bacc
    import concourse.tile as tile
    from concourse import mybir
    from concourse.masks import make_identity
    import concourse.bass as bass

    f32 = mybir.dt.float32
    bf16 = mybir.dt.bfloat16
    f16 = mybir.dt.float16
    Alu = mybir.AluOpType
    Act = mybir.ActivationFunctionType

    nc = bacc.Bacc("TRN2", target_bir_lowering=False, debug=False,
                   num_devices=N_CORES)

    # Per-core DRAM I/O (layouts chosen so every DMA has long contiguous
    # runs).
    xt = nc.dram_tensor("xt", [128, NTILES, K_CH, TT], f16,
                        kind="ExternalInput")
    # bf16 correction operands: [...,0,...] = eps_x, [...,1,...] = x
    xc = nc.dram_tensor("xc", [128, NTILES, 2, K_CH, TT], bf16,
                        kind="ExternalInput")
    w1 = nc.dram_tensor("w1", [128, K_CH, WIDTH], f16, kind="ExternalInput")
    # corr weights: [...,0,:] = bf16(w16[:, :CORR]), [...,1,:] = bf16(eps_w)
    w1c = nc.dram_tensor("w1c", [128, K_CH, 2, CORR], bf16,
                         kind="ExternalInput")
    b1 = nc.dram_tensor("b1", [WIDTH], f32, kind="ExternalInput")
    w2 = nc.dram_tensor("w2", [128, C_CH, DIM], f16, kind="ExternalInput")
    y = nc.dram_tensor("y", [TOK_PER_CORE, DIM], f32, kind="ExternalOutput")

    with tile.TileContext(nc) as tc:
        with (
            tc.tile_pool(name="wts", bufs=1) as wts,
            tc.tile_pool(name="xts", bufs=3) as xts,
            tc.tile_pool(name="logits", bufs=2) as logits_pool,
            tc.tile_pool(name="mask", bufs=2) as mask_pool,
            tc.tile_pool(name="acts", bufs=2) as acts_pool,
            tc.tile_pool(name="out", bufs=2) as out_pool,
            tc.tile_pool(name="pl", bufs=1, space="PSUM") as pl_pool,
            tc.tile_pool(name="pt", bufs=2, space="PSUM") as pt_pool,
            tc.tile_pool(name="py", bufs=2, space="PSUM") as py_pool,
        ):
            # ---- resident weights (DMAs emitted in need-order below) ----
            w1_sb = wts.tile([128, K_CH, WIDTH], f16)
            w1c_sb = wts.tile([128, K_CH, 2, CORR], bf16)
            w2_sb = wts.tile([128, C_CH, DIM], f16)
            b1_sb = wts.tile([128, WIDTH], f32)
            ident = wts.tile([128, 128], f16)

            xt_tiles = {}

            def prefetch_xt(j, eng=None):
                xm = xts.tile([128, K_CH, TT], f16, tag="x")
                xcc = xts.tile([128, 2, K_CH, TT], bf16, tag="xc")
                (eng or nc.sync).dma_start(out=xm, in_=xt[:, j, :, :])
                (eng or nc.sync).dma_start(out=xcc, in_=xc[:, j, :, :, :])
                xt_tiles[j] = (xm, xcc)

            # Weight DMAs chunked and emitted on the Sync engine in
            # consumption order (only Sync's HW DGE fans out over all 16
            # queues, ~400 GB/s; each dma_start dispatch costs ~0.6us).
            # x-tile prefetches ride GpSimd's slower SW DGE — their
            # deadlines are generous and this keeps Sync free for weights.
            nc.sync.dma_start(out=w1_sb[:, 0, :], in_=w1[:, 0, :])
            xm0 = xts.tile([128, K_CH, TT], f16, tag="x")
            xcc0 = xts.tile([128, 2, K_CH, TT], bf16, tag="xc")
            nc.sync.dma_start(out=xm0, in_=xt[:, 0, :, :])
            xt_tiles[0] = (xm0, xcc0)
            nc.sync.dma_start(out=w1_sb[:, 1, :], in_=w1[:, 1, :])
            for k in range(2, K_CH, 2):
                nc.sync.dma_start(out=w1_sb[:, k:k + 2, :],
                                  in_=w1[:, k:k + 2, :])
            nc.sync.dma_start(out=w1c_sb, in_=w1c[:, :, :, :])
            # xcc0 is never consumed (tile 0 skips the correction pass) —
            # no DMA for it.
            prefetch_xt(1, nc.gpsimd)
            for c in range(0, C_CH, 4):
                nc.sync.dma_start(out=w2_sb[:, c:c + 4, :],
                                  in_=w2[:, c:c + 4, :])
            # bias broadcast rides GpSimd's SW DGE, off the weight path
            b1_bcast = bass.AP(tensor=b1, offset=0, ap=[[0, 128], [1, WIDTH]])
            nc.gpsimd.dma_start(out=b1_sb, in_=b1_bcast)
            make_identity(nc, ident)

            # per-token-tile masked-acts, produced by stage A (GEMM1+mask),
            # consumed by stage B (transpose + GEMM2); 1-deep software
            # pipeline so the PE never waits on the vector-engine epilogue.
            state = {}

            def stage_a(j):
                if j not in xt_tiles:
                    prefetch_xt(j, nc.gpsimd)
                xm, xcc = xt_tiles.pop(j)
                do_corr = j > 0

                lg = logits_pool.tile([TT, WIDTH], f32, tag="lg")
                d1 = mask_pool.tile([TT, DEC_COLS], f16, tag="d1")
                vv = mask_pool.tile([TT, WIDTH], f16, tag="vv")
                ac = acts_pool.tile([TT, WIDTH], f16, tag="ac")
                mk = acts_pool.tile([TT, NODES_PAD], f16, tag="mk")

                # main fp16 pass, k-outer so the PE can start as soon as
                # the first w1 k-chunk lands
                pls = [pl_pool.tile([TT, NT_W], f32, name=f"pl{nt}")
                       for nt in range(4)]
                for k in range(K_CH):
                    for nt in range(4):
                        nc.tensor.matmul(
                            pls[nt], lhsT=xm[:, k, :],
                            rhs=w1_sb[:, k, nt * NT_W:(nt + 1) * NT_W],
                            start=(k == 0),
                            stop=(k == K_CH - 1 and not (nt == 0 and do_corr)))
                # bf16 correction into the nt0 PSUM group (levels 0..4)
                if do_corr:
                    for p in range(2):
                        for k in range(K_CH):
                            nc.tensor.matmul(
                                pls[0][:, 0:CORR], lhsT=xcc[:, p, k, :],
                                rhs=w1c_sb[:, k, p, :], start=False,
                                stop=(p == 1 and k == K_CH - 1))

                for nt in (1, 2, 3, 0):
                    nsl = slice(nt * NT_W, (nt + 1) * NT_W)
                    # bias add (fp32, exact) PSUM -> SBUF
                    nc.vector.tensor_tensor(lg[:, nsl], pls[nt],
                                            b1_sb[:, nsl], Alu.add)
                    if nt == 0:
                        nc.vector.tensor_scalar(
                            d1[:, 0:NT_W], lg[:, 0:NT_W], 0.0, None,
                            Alu.is_gt)
                    elif nt == 1:
                        nc.vector.tensor_scalar(
                            d1[:, NT_W:DEC_COLS], lg[:, NT_W:DEC_COLS],
                            0.0, None, Alu.is_gt)
                    nc.scalar.activation(ac[:, nsl], lg[:, nsl], Act.Silu)

                # tree mask: V_0 = 1 at root cols; then per level
                # child1 = V_d * dec_d, child0 = V_d - child1
                nc.vector.memset(vv[:, 0:8], 1.0)
                for d in range(DEPTH):
                    ld = 8 * (1 << d)
                    c0 = 8 * ((1 << d) - 1)
                    c1 = 8 * ((1 << (d + 1)) - 1)
                    vpar = vv[:, c0:c0 + ld].rearrange("p (i t) -> p i t", t=8)
                    dpar = d1[:, c0:c0 + ld].rearrange("p (i t) -> p i t", t=8)
                    kids = vv[:, c1:c1 + 2 * ld].rearrange(
                        "p (i two t) -> p i two t", two=2, t=8)
                    nc.vector.tensor_tensor(kids[:, :, 1, :], vpar, dpar,
                                            Alu.mult)
                    nc.vector.tensor_tensor(kids[:, :, 0, :], vpar,
                                            kids[:, :, 1, :], Alu.subtract)

                # masked acts (fp16); cols 2040:2048 are zero padding so the
                # last transpose/GEMM2 chunk is a uniform 128 wide
                nc.vector.memset(mk[:, WIDTH:NODES_PAD], 0.0)
                nc.vector.tensor_tensor(mk[:, 0:1024], ac[:, 0:1024],
                                        vv[:, 0:1024], Alu.mult)
                nc.vector.tensor_tensor(mk[:, 1024:WIDTH], ac[:, 1024:WIDTH],
                                        vv[:, 1024:WIDTH], Alu.mult)
                state[j] = mk

            def stage_b(j):
                mk = state.pop(j)
                at = acts_pool.tile([128, C_CH, TT], f16, tag="at")
                # transpose in groups -> one PSUM tile -> one copy; first
                # group is a single chunk so GEMM2 can start immediately
                c = 0
                for gsz in (1, 3, 4, 4, 4):
                    pt = pt_pool.tile([128, 512], f16)
                    for i in range(gsz):
                        nc.tensor.transpose(
                            pt[:, i * 128:(i + 1) * 128],
                            mk[:, (c + i) * 128:(c + i + 1) * 128], ident)
                    nc.scalar.copy(
                        at[:, c:c + gsz, :],
                        pt[:, :gsz * 128].rearrange("p (c t) -> p c t", t=TT))
                    c += gsz
                ys = out_pool.tile([TT, DIM], f32, tag="ys")
                for h in range(2):
                    hs = slice(h * 512, (h + 1) * 512)
                    py = py_pool.tile([TT, 512], f32)
                    for c in range(C_CH):
                        nc.tensor.matmul(
                            py, lhsT=at[:, c, :], rhs=w2_sb[:, c, hs],
                            start=(c == 0), stop=(c == C_CH - 1))
                    nc.vector.tensor_copy(ys[:, hs], py)
                    nc.sync.dma_start(out=y[j * TT:(j + 1) * TT, hs],
                                      in_=ys[:, hs])

            # software pipeline: A(0), A(1), B(0), A(2), B(1), ... B(7)
            stage_a(0)
            for j in range(1, NTILES):
                stage_a(j)
                stage_b(j - 1)
            stage_b(NTILES - 1)

    nc.finalize()
    return nc


def _get_program():
    global _PROGRAM
    if _PROGRAM is None:
        _PROGRAM = _build_program()
    return _PROGRAM


def kernel(oldx, W_in, b_in, W_out):
    from concourse.bass_utils import run_bass_kernel_spmd

    bf16 = ml_dtypes.bfloat16
    oldx = np.asarray(oldx)
    W_in = np.asarray(W_in, dtype=np.float32)
    b_in = np.asarray(b_in, dtype=np.float32)
    W_out = np.asarray(W_out, dtype=np.float32)
    x = oldx.reshape(-1, DIM).astype(np.float32)          # [8192, 1024]

    # node-major column permutation: our col 8n+t  <-  ref col 255t+n
    i = np.arange(WIDTH)
    perm = 255 * (i % PAR) + (i // PAR)

    w1t = W_in[perm, :].T.astype(np.float32)              # [1024, 2040]
    w16 = w1t.astype(np.float16)
    # [dim, width] -> [128, K_CH, WIDTH] with dim = k*128 + p
    w1 = np.ascontiguousarray(
        w16.reshape(K_CH, 128, WIDTH).transpose(1, 0, 2))
    # corr weights for cols 0..CORR: [128, K_CH, 2, CORR]
    wb = w16[:, :CORR].astype(np.float32).astype(bf16)
    ewb = (w1t - w16.astype(np.float32))[:, :CORR].astype(bf16)
    w1c = np.ascontiguousarray(
        np.stack([wb, ewb], axis=1).reshape(K_CH, 128, 2, CORR)
        .transpose(1, 0, 2, 3))
    b1 = np.ascontiguousarray(b_in[perm])

    w2t = np.zeros((NODES_PAD, DIM), np.float32)
    w2t[:WIDTH] = W_out.T[perm, :]
    w2 = np.ascontiguousarray(
        w2t.astype(np.float16).reshape(C_CH, 128, DIM).transpose(1, 0, 2))

    in_maps = []
    for c in range(N_CORES):
        xcf = x[c * TOK_PER_CORE:(c + 1) * TOK_PER_CORE]  # [1024, 1024]
        xT = xcf.T                                        # [dim, tok] f32
        x16 = xT.astype(np.float16)
        ex = (xT - x16.astype(np.float32)).astype(bf16)
        xb = x16.astype(np.float32).astype(bf16)
        # [dim, tok] -> [128, NTILES, K_CH, TT]; dim = k*128+p, tok = j*128+t
        def lay(a):
            return a.reshape(K_CH, 128, NTILES, TT).transpose(1, 2, 0, 3)
        xtc = np.ascontiguousarray(lay(x16))
        xcc = np.ascontiguousarray(
            np.stack([lay(ex), lay(xb)], axis=2))
        in_maps.append({
            "xt": xtc, "xc": xcc, "w1": w1, "w1c": w1c,
            "b1": b1, "w2": w2,
        })

    nc = _get_program()
    res = run_bass_kernel_spmd(nc, in_maps, core_ids=list(range(N_CORES)))
    out = np.concatenate([res.results[c]["y"] for c in range(N_CORES)],
                         axis=0)
    return out.reshape(oldx.shape).astype(np.float32)
